# revision 3
# baseline (speedup 1.0000x reference)
"""CLSTMCell fused cell kernel for 8 Trainium2 NeuronCores.

Data-parallel over the batch: each of the 8 cores processes a 512-row batch
shard; the four (D,4U) kernels and biases are replicated to every core.

Math (per batch shard, D = U = 1024):
    zr = xr@R + xi@I + hr@Rr + hi@Ir + br          [512, 4096]
    zi = xi@R - xr@I + hi@Rr - hr@Ir + bi          [512, 4096]
    per gate g (i,f,c,o) and half v (r from zr, i from zi):
        i,f,o -> hard_sigmoid(z) = clip(0.2 z + 0.5, 0, 1);  c~ -> tanh(z)
    c = f*c_tm1 + i*tanh(c~);  h = o*tanh(c)

Device layout: batch rows on SBUF partitions for all elementwise math. The
matmuls contract over k (4096 = 32 blocks of 128) with the transposed
activation block [128k, 128b] stationary and a [128k, 256n] weight tile
moving at float32r (full-rate fp32). Host-side work is layout only: slicing
the batch shard, transposing activations to [k, b], and replicating biases
across partitions. zi reuses the same weight tiles as zr by negating the
xr/hr activation blocks once on device.
"""

import sys

sys.path.insert(0, "/opt/trn_rl_repo")

import numpy as np

import concourse.bacc as bacc
import concourse.mybir as mybir
import concourse.tile as tile
from concourse.bass_utils import run_bass_kernel_spmd

N_CORES = 8
B, D, U = 4096, 1024, 1024
BS = B // N_CORES          # batch rows per core
P = 128                    # SBUF partitions
KB = (2 * D + 2 * U) // P  # 32 contraction blocks of 128
NB = BS // P               # 4 batch tiles per core
NCHUNK = 256               # psum free width (1 bank even at fp32, N>=256 for f32r)
F32 = mybir.dt.float32
F32R = mybir.dt.float32r
# gate order: f first (starts c accumulation), then c~, i (finishes c), o
GATE_ORDER = (1, 2, 0, 3)

_CACHE = {}


def _build():
    nc = bacc.Bacc("TRN2", target_bir_lowering=False, debug=False,
                   num_devices=N_CORES)

    din = {}
    for name in ("xrT", "xiT", "hrT", "hiT"):
        din[name] = nc.dram_tensor(name, [D, BS], F32R, kind="ExternalInput").ap()
    din["c_prev"] = nc.dram_tensor("c_prev", [BS, 2 * U], F32,
                                   kind="ExternalInput").ap()
    for name in ("wr", "wi", "wrr", "wir"):
        din[name] = nc.dram_tensor(name, [D, 4 * U], F32R,
                                   kind="ExternalInput").ap()
    din["brep_r"] = nc.dram_tensor("brep_r", [P, 4 * U], F32,
                                   kind="ExternalInput").ap()
    din["brep_i"] = nc.dram_tensor("brep_i", [P, 4 * U], F32,
                                   kind="ExternalInput").ap()
    h_out = nc.dram_tensor("h_out", [BS, 2 * U], F32, kind="ExternalOutput").ap()
    c_out = nc.dram_tensor("c_out", [BS, 2 * U], F32, kind="ExternalOutput").ap()

    with tile.TileContext(nc) as tc:
        with (
            tc.tile_pool(name="acts", bufs=48) as acts,
            tc.tile_pool(name="brep", bufs=2) as breps,
            tc.tile_pool(name="wpool", bufs=8) as wpool,
            tc.tile_pool(name="cprev", bufs=9) as cpool,
            tc.tile_pool(name="cacc", bufs=8) as cacc_p,
            tc.tile_pool(name="tct", bufs=8) as tct_p,
            tc.tile_pool(name="tc2", bufs=8) as tc2_p,
            tc.tile_pool(name="gat", bufs=4) as gat_p,
            tc.tile_pool(name="tmp", bufs=4) as tmp_p,
            tc.tile_pool(name="outs", bufs=6) as out_p,
            tc.tile_pool(name="psum", bufs=8, space="PSUM") as psum_p,
        ):
            # --- resident activation blocks, [128k, 512b] each -------------
            def load_act(src):
                tiles = []
                for j in range(D // P):
                    t = acts.tile([P, BS], F32R, tag="acts")
                    nc.sync.dma_start(t[:], src[j * P:(j + 1) * P, :])
                    tiles.append(t)
                return tiles

            xr = load_act(din["xrT"])
            xi = load_act(din["xiT"])
            hr = load_act(din["hrT"])
            hi = load_act(din["hiT"])

            def negate(blocks):
                out = []
                for src in blocks:
                    t = acts.tile([P, BS], F32R, tag="acts")
                    nc.vector.tensor_scalar_mul(t[:], src[:], -1.0)
                    out.append(t)
                return out

            nxr = negate(xr)
            nhr = negate(hr)

            a_blocks = xr + xi + hr + hi        # zr stationary blocks
            b_blocks = xi + nxr + hi + nhr      # zi stationary blocks
            wsrc = [din["wr"]] * 8 + [din["wi"]] * 8 \
                + [din["wrr"]] * 8 + [din["wir"]] * 8
            wrow = [(k % 8) * P for k in range(KB)]

            # --- biases, replicated across partitions ----------------------
            # hard-sigmoid gates (i,f,o) get the affine fold 0.2*b + 0.5 so the
            # gate epilogue is (z*0.2 + brep); the c~ columns keep the raw bias.
            brep = []
            for name in ("brep_r", "brep_i"):
                t = breps.tile([P, 4 * U], F32, tag="brep")
                nc.sync.dma_start(t[:], din[name][:, :])
                nc.vector.tensor_scalar(t[:, 0:2 * U], t[:, 0:2 * U],
                                        0.2, 0.5, mybir.AluOpType.mult,
                                        mybir.AluOpType.add)
                nc.vector.tensor_scalar(t[:, 3 * U:4 * U], t[:, 3 * U:4 * U],
                                        0.2, 0.5, mybir.AluOpType.mult,
                                        mybir.AluOpType.add)
                brep.append(t)

            # --- main loop --------------------------------------------------
            n_uc = U // NCHUNK  # 4 column chunks of 256 within a gate
            for uc in range(n_uc):
                # c_tm1 tiles for this chunk: z=0 real cols, z=1 imag cols
                cprev = {}
                for b in range(NB):
                    for z in range(2):
                        col0 = z * U + uc * NCHUNK
                        t = cpool.tile([P, NCHUNK], F32, tag="cprev")
                        nc.sync.dma_start(
                            t[:], din["c_prev"][b * P:(b + 1) * P,
                                                col0:col0 + NCHUNK])
                        cprev[(b, z)] = t

                cacc = {}
                tct = {}
                tc2 = {}
                for g in GATE_ORDER:
                    n0 = g * U + uc * NCHUNK
                    # 8 live psum accumulation groups, interleaved over k so
                    # each weight tile is consumed by 8 back-to-back matmuls
                    ps = {(b, z): psum_p.tile([P, NCHUNK], F32, tag="ps",
                                              name=f"ps_{uc}_{g}_{b}_{z}")
                          for b in range(NB) for z in range(2)}
                    for k in range(KB):
                        wt = wpool.tile([P, NCHUNK], F32R, tag="w")
                        nc.sync.dma_start(
                            wt[:], wsrc[k][wrow[k]:wrow[k] + P,
                                           n0:n0 + NCHUNK])
                        wtr = wt[:]
                        for b in range(NB):
                            for z in range(2):
                                blk = (a_blocks if z == 0 else b_blocks)[k]
                                nc.tensor.matmul(
                                    ps[(b, z)][:],
                                    blk[:, b * P:(b + 1) * P],
                                    wtr,
                                    start=(k == 0), stop=(k == KB - 1))

                    # epilogue per group
                    for b in range(NB):
                        for z in range(2):
                            p = ps[(b, z)]
                            bia = brep[z][:, n0:n0 + NCHUNK]
                            if g == 2:  # c~ : tanh(z + b)
                                t = tmp_p.tile([P, NCHUNK], F32, tag="tmp")
                                nc.vector.tensor_tensor(
                                    t[:], p[:], bia, mybir.AluOpType.add)
                                tt = tct_p.tile([P, NCHUNK], F32, tag="tct")
                                nc.scalar.activation(
                                    tt[:], t[:],
                                    mybir.ActivationFunctionType.Tanh)
                                tct[(b, z)] = tt
                                continue
                            # hard-sigmoid gate: clip(z*0.2 + brep, 0, 1)
                            gt = gat_p.tile([P, NCHUNK], F32, tag="gat")
                            nc.vector.scalar_tensor_tensor(
                                gt[:], p[:], 0.2, bia,
                                mybir.AluOpType.mult, mybir.AluOpType.add)
                            nc.vector.tensor_scalar(
                                gt[:], gt[:], 1.0, 0.0,
                                mybir.AluOpType.min, mybir.AluOpType.max)
                            col0 = z * U + uc * NCHUNK
                            rows = slice(b * P, (b + 1) * P)
                            if g == 1:  # f: start c accumulation
                                ca = cacc_p.tile([P, NCHUNK], F32, tag="cacc")
                                nc.vector.tensor_tensor(
                                    ca[:], gt[:], cprev[(b, z)][:],
                                    mybir.AluOpType.mult)
                                cacc[(b, z)] = ca
                            elif g == 0:  # i: finish c, emit tanh(c)
                                t = tmp_p.tile([P, NCHUNK], F32, tag="tmp")
                                nc.vector.tensor_tensor(
                                    t[:], gt[:], tct[(b, z)][:],
                                    mybir.AluOpType.mult)
                                cn = out_p.tile([P, NCHUNK], F32, tag="out")
                                nc.vector.tensor_tensor(
                                    cn[:], t[:], cacc[(b, z)][:],
                                    mybir.AluOpType.add)
                                nc.sync.dma_start(
                                    c_out[rows, col0:col0 + NCHUNK], cn[:])
                                t2 = tc2_p.tile([P, NCHUNK], F32, tag="tc2")
                                nc.scalar.activation(
                                    t2[:], cn[:],
                                    mybir.ActivationFunctionType.Tanh)
                                tc2[(b, z)] = t2
                            else:  # o: h = o * tanh(c)
                                ht = out_p.tile([P, NCHUNK], F32, tag="out")
                                nc.vector.tensor_tensor(
                                    ht[:], gt[:], tc2[(b, z)][:],
                                    mybir.AluOpType.mult)
                                nc.sync.dma_start(
                                    h_out[rows, col0:col0 + NCHUNK], ht[:])

    nc.compile()
    return nc


def kernel(inputs, h_tm1, c_tm1, real_kernel, imaginary_kernel,
           real_recurrent_kernel, imaginary_recurrent_kernel,
           real_bias, imaginary_bias):
    if "nc" not in _CACHE:
        _CACHE["nc"] = _build()
    nc = _CACHE["nc"]

    inputs = np.ascontiguousarray(inputs, dtype=np.float32)
    h_tm1 = np.ascontiguousarray(h_tm1, dtype=np.float32)
    c_tm1 = np.ascontiguousarray(c_tm1, dtype=np.float32)
    wr = np.ascontiguousarray(real_kernel, dtype=np.float32)
    wi = np.ascontiguousarray(imaginary_kernel, dtype=np.float32)
    wrr = np.ascontiguousarray(real_recurrent_kernel, dtype=np.float32)
    wir = np.ascontiguousarray(imaginary_recurrent_kernel, dtype=np.float32)
    brep_r = np.ascontiguousarray(
        np.broadcast_to(np.asarray(real_bias, np.float32), (P, 4 * U)))
    brep_i = np.ascontiguousarray(
        np.broadcast_to(np.asarray(imaginary_bias, np.float32), (P, 4 * U)))

    in_maps = []
    for c in range(N_CORES):
        rows = slice(c * BS, (c + 1) * BS)
        in_maps.append({
            "xrT": np.ascontiguousarray(inputs[rows, :D].T),
            "xiT": np.ascontiguousarray(inputs[rows, D:].T),
            "hrT": np.ascontiguousarray(h_tm1[rows, :U].T),
            "hiT": np.ascontiguousarray(h_tm1[rows, U:].T),
            "c_prev": np.ascontiguousarray(c_tm1[rows]),
            "wr": wr, "wi": wi, "wrr": wrr, "wir": wir,
            "brep_r": brep_r, "brep_i": brep_i,
        })

    res = run_bass_kernel_spmd(nc, in_maps, list(range(N_CORES)))
    h = np.concatenate([res.results[c]["h_out"] for c in range(N_CORES)], axis=0)
    c = np.concatenate([res.results[c]["c_out"] for c in range(N_CORES)], axis=0)
    return h, c
